# revision 20
# baseline (speedup 1.0000x reference)
"""Self-contained Trainium2 Bass kernel for the CharRNN problem:
2-layer LSTM (B=32, T=256, H=256) + V=32000 softmax cross-entropy mean loss.

Strategy (8 NeuronCores, SPMD):
  * the LSTM recurrence is replicated on every core (latency-bound)
  * the softmax matmul + exp is sharded over the vocab: each core owns a
    4000-wide shard of softmax_w, computes logits for all 8192 rows against
    its shard, reduces them to per-row sum(exp(logit)) plus the per-row
    target logit; the host combines loss_r = log(sum_c se_r) - tgt_logit_r

Device-side structure (v4 — transposed "zT" gate layout):
  * wavefront: slot t runs L1 step t and L2 step t-1 so the two layer
    recurrence chains interleave on the engines
  * gates are computed TRANSPOSED: z^T chunks [128(gate), 32(batch)] via
    W-stationary matmuls (lhsT = W 128x128 chunk, rhs = hidden-major
    h/x k-tiles).  h comes OUT of the cell ops already hidden-major
    [128, 2, 32] -> no transposes at all, and every elementwise/ACT op
    runs on all 128 partitions (4x lane utilization vs the [32, *] form)
  * the x-part of L1's gates is batched over a 2-step window into the
    same psum the per-step h-part accumulates into (saves ~2K PE
    rows/step of W streaming)
  * cell state kept as C' = 2c: Pool computes t1' = (g_i+1)*g_j, DVE
    computes sig(f)*C' and the add, ACT computes tanh(C' * 0.5) using
    the activation input scale; gate col order [i, o, j, f] with the 0.5
    sigmoid input scale folded into W on the host; per-partition biases
    (forget +0.5) are added by K=1 matmuls (lhsT = bias row, rhs = ones)
  * exp over PAIRS of 500-wide vocab chunks ([128,2,500] strided AP);
    per slot the previous slot's exp is emitted FIRST on ACT (its psum is
    ready at slot start), then the logits MMs (PE fill), then the gates
"""
import numpy as np
import ml_dtypes
import concourse.bass as bass
import concourse.mybir as mybir
import concourse.tile as tile
from concourse import bacc
from concourse.bass_utils import run_bass_kernel_spmd

F32 = mybir.dt.float32
BF16 = mybir.dt.bfloat16
F8 = mybir.dt.float8e4
I32 = mybir.dt.int32
I16 = mybir.dt.int16
AF = mybir.ActivationFunctionType
ALU = mybir.AluOpType
DR = mybir.MatmulPerfMode.DoubleRow
NPF8 = ml_dtypes.float8_e4m3fn

B, T, H, V, NCORES = 32, 256, 256, 32000, 8


def build_charrnn(T=256, V=32000, n_cores=8, has_b1=False, has_b2=False,
                  has_swb=False, num_devices=8):
    B, H = 32, 256
    G4 = 4 * H                      # 1024 gate width
    VS = V // n_cores               # vocab shard per core
    BT = B * T
    RT = BT // 128                  # 128-row tiles (4 steps each)
    assert T % 4 == 0 and BT % 128 == 0

    # one psum BANK per matmul chunk (a matmul may not cross a bank)
    CH = max(d for d in range(1, 513) if VS % d == 0)   # 500
    NCHUNK = VS // CH                                    # 8
    NPAIR = NCHUNK // 2                                  # 4 exp calls per tile

    nc = bacc.Bacc("TRN2", target_bir_lowering=False, debug=False,
                   num_devices=num_devices)

    # ---------------- DRAM I/O ----------------
    ids_d = nc.dram_tensor("ids", (RT, 128, 1), I32, kind="ExternalInput")
    emb_d = nc.dram_tensor("emb", (V, H), F32, kind="ExternalInput")
    w1_d = nc.dram_tensor("w1", (4, 128, G4), F8, kind="ExternalInput")
    w2_d = nc.dram_tensor("w2", (4, 128, G4), F8, kind="ExternalInput")
    sw_d = nc.dram_tensor("sw", (2, 128, VS), F8, kind="ExternalInput")
    swp_d = nc.dram_tensor("swp", (2, 128, VS, 2), I16, kind="ExternalInput")
    tgi_d = nc.dram_tensor("tgi", (RT, 128, 8), I16, kind="ExternalInput")
    # combined per-gate bias rows (perm+scaled b + forget +0.5), always fed
    b1x_d = nc.dram_tensor("b1x", (1, G4), BF16, kind="ExternalInput")
    b2x_d = nc.dram_tensor("b2x", (1, G4), BF16, kind="ExternalInput")
    if has_swb:
        swb_d = nc.dram_tensor("swbp", (128, VS), F32, kind="ExternalInput")
    se_d = nc.dram_tensor("se_out", (128, RT * NPAIR), F32,
                          kind="ExternalOutput")
    tg_d = nc.dram_tensor("tg_out", (1, BT), F32, kind="ExternalOutput")

    with tile.TileContext(nc) as tc:
        with tc.tile_pool(name="persist", bufs=1) as pp:
            # ---- persistent SBUF ----
            w1_sb = pp.tile([128, 4, G4], F8, tag="w1")
            w2_sb = pp.tile([128, 4, G4], F8, tag="w2")
            nc.sync.dma_start(w1_sb[:], w1_d[:].rearrange("k p c -> p k c"))
            nc.sync.dma_start(w2_sb[:], w2_d[:].rearrange("k p c -> p k c"))
            sw_sb = pp.tile([128, 2, VS], F8, tag="sw")
            nc.sync.dma_start(sw_sb[:], sw_d[:].rearrange("k p c -> p k c"))
            swp_sb = pp.tile([128, 2, VS, 2], I16, tag="swp")
            nc.sync.dma_start(swp_sb[:],
                              swp_d[:].rearrange("k p c d -> p k c d"))
            # h2 slab: fp8 for the logits/recurrence matmuls plus a bf16
            # copy (exact upcast of the same fp8 values) for the
            # target-logit DVE multiply
            hs = pp.tile([128, 2, BT], BF16, tag="hs")
            hs8 = pp.tile([128, 2, BT], F8, tag="hs8")

            ones_bf = pp.tile([128, 1], BF16, tag="ones")
            nc.gpsimd.memset(ones_bf[:], 1.0)
            # ones row for K=1 per-partition bias matmuls (rhs side)
            ones_row = pp.tile([1, 64], BF16, tag="onesrow")
            nc.gpsimd.memset(ones_row[:], 1.0)
            # per-gate bias rows (lhsT side of the K=1 bias matmuls)
            b1x_sb = pp.tile([1, G4], BF16, tag="b1x")
            b2x_sb = pp.tile([1, G4], BF16, tag="b2x")
            nc.sync.dma_start(b1x_sb[:], b1x_d[:])
            nc.sync.dma_start(b2x_sb[:], b2x_d[:])
            # bias chunks: all 8 if a real b was provided, else just the
            # forget-gate chunks 6,7 (+0.5)
            bch1 = range(8) if has_b1 else range(6, 8)
            bch2 = range(8) if has_b2 else range(6, 8)

            # cell state, kept as C' = 2c, hidden-major [128, 2, 32]
            c1 = pp.tile([128, 2, 32], F32, tag="c1")
            c2 = pp.tile([128, 2, 32], F32, tag="c2")
            nc.gpsimd.memset(c1[:], 0.0)
            nc.gpsimd.memset(c2[:], 0.0)
            # zero h for the step-0 h-part matmuls
            h0T = pp.tile([128, 2, 32], F8, tag="h0T")
            nc.gpsimd.memset(h0T[:], 0.0)

            se_sb = pp.tile([128, RT * NPAIR], F32, tag="se")
            tg_sb = pp.tile([1, BT], F32, tag="tg")
            # accum_out adds into existing SBUF content on HW — zero it
            nc.gpsimd.memset(se_sb[:], 0.0)

            if has_swb:
                swb_sb = pp.tile([128, VS], F32, tag="swb")
                nc.sync.dma_start(swb_sb[:], swb_d[:])

            # ============ fused phase: gather + LSTM + logits ============
            with (
                tc.tile_pool(name="xsp", bufs=1) as xsp,
                tc.tile_pool(name="stage", bufs=8) as stp,
                tc.tile_pool(name="lwork", bufs=3) as lw,
                tc.tile_pool(name="xwp", bufs=2, space="PSUM") as xwp,
                tc.tile_pool(name="zp", bufs=2, space="PSUM") as zp,
                tc.tile_pool(name="ep", bufs=2, space="PSUM") as ep,
                tc.tile_pool(name="ework", bufs=3) as ew,
            ):
                xs = xsp.tile([128, 2, BT], BF16, tag="xs")
                xs8 = xsp.tile([128, 2, BT], F8, tag="xs8")

                # ---- embedding gather (time-major) + transpose to slabs;
                # emitted incrementally from the slot loop so the engine
                # queues are ordered to match data arrival ----
                def emit_gather(rt):
                    ids_sb = stp.tile([128, 1], I32, tag="ids")
                    nc.gpsimd.dma_start(ids_sb[:], ids_d.ap()[rt])
                    xrow = stp.tile([128, H], F32, tag="xrow")
                    nc.gpsimd.indirect_dma_start(
                        out=xrow[:], out_offset=None,
                        in_=emb_d[:],
                        in_offset=bass.IndirectOffsetOnAxis(
                            ap=ids_sb[:, :1], axis=0),
                    )
                    xbf = stp.tile([128, H], BF16, tag="xbf")
                    nc.gpsimd.tensor_copy(xbf[:], xrow[:])
                    cs = 128 * rt
                    nc.sync.dma_start_transpose(
                        xs[:, 0, cs:cs + 128], xbf[:, 0:128])
                    nc.sync.dma_start_transpose(
                        xs[:, 1, cs:cs + 128], xbf[:, 128:256])
                    nc.gpsimd.tensor_copy(xs8[:, :, cs:cs + 128],
                                          xs[:, :, cs:cs + 128])

                for rt in range(6):
                    emit_gather(rt)

                def emit_logits_mms(rt, p):
                    """Logits matmuls for vocab chunks (2p, 2p+1) of row-tile
                    rt; p==3 also emits the target-logit gather+reduce.
                    Returns state for the deferred exp/copy emission."""
                    cs = 128 * rt
                    pse = ep.tile([128, 2, 512], F32, tag="pse")
                    for half, c in enumerate((2 * p, 2 * p + 1)):
                        nc.tensor.matmul(
                            pse[:, half, 0:CH], hs8[:, :, cs:cs + 128],
                            sw_sb[:, :, c * CH:c * CH + CH],
                            start=True, stop=True, perf_mode=DR,
                        )
                        if has_swb:
                            nc.vector.tensor_tensor(
                                out=pse[:, half, 0:CH], in0=pse[:, half, 0:CH],
                                in1=swb_sb[:, (2 * p + half) * CH:
                                           (2 * p + half) * CH + CH],
                                op=ALU.add)
                    pst = None
                    if p == 3:
                        tgi_sb = ew.tile([128, 8], I16, tag="tgi")
                        nc.gpsimd.dma_start(tgi_sb[:], tgi_d.ap()[rt])
                        pstt = ep.tile([128, 2, 512], F32, tag="pse")
                        pst = pstt[0:1, 0, 0:128]
                        for k in range(2):
                            swg = ew.tile([128, 128, 2], I16, tag="swg")
                            nc.gpsimd.ap_gather(
                                swg[:], swp_sb[:, k], tgi_sb[:],
                                channels=128, num_elems=VS, d=2, num_idxs=128,
                            )
                            mulk = ew.tile([128, 128], BF16, tag="mulk")
                            nc.vector.tensor_tensor(
                                out=mulk[:],
                                in0=swg[:].bitcast(BF16)[:, :, 0],
                                in1=hs[:, k, cs:cs + 128],
                                op=ALU.mult)
                            nc.tensor.matmul(pst, ones_bf[:, 0:1], mulk[:],
                                             start=(k == 0), stop=(k == 1))
                    return pse, pst, rt, p, cs

                def emit_exp(state):
                    pse, pst, rt, p, cs = state
                    ebuf = ew.tile([128, 2, CH], BF16, tag="ebuf")
                    nc.scalar.activation(ebuf[:], pse[:, :, 0:CH], AF.Exp)
                    # exp row-sum on DVE (frees the ACT accumulator read)
                    nc.vector.tensor_reduce(
                        se_sb[:, rt * NPAIR + p:rt * NPAIR + p + 1], ebuf[:],
                        mybir.AxisListType.XY, ALU.add)
                    if pst is not None:
                        nc.scalar.copy(tg_sb[0:1, cs:cs + 128], pst)

                def emit_xwindow(m):
                    """L1 x-part + bias MMs for steps (2m, 2m+1) into a
                    fresh [128, 8chunk, 2step, 32batch] psum (one bank).
                    The bias matmuls OPEN their chunks' accumulation; the
                    per-step h-part closes each step's column slice."""
                    t0 = 2 * m
                    xw = xwp.tile([128, 8, 2, 32], F32, tag="xw")
                    for c in bch1:
                        nc.tensor.matmul(
                            xw[:, c, :, :],
                            b1x_sb[0:1, c * 128:(c + 1) * 128],
                            ones_row[0:1, 0:64], start=True, stop=False)
                    for c in range(8):
                        nc.tensor.matmul(
                            xw[:, c, :, :],
                            w1_sb[:, 0:2, c * 128:(c + 1) * 128],
                            xs8[:, :, 32 * t0:32 * t0 + 64],
                            start=(c not in bch1), stop=False,
                            perf_mode=DR)
                    return xw

                def emit_l1_h(xw, t, h1T_tm1):
                    """L1(t) h-part: W-stationary DoubleRow matmuls (K=256
                    in one pass) into this step's x-window psum slice."""
                    sl = t % 2
                    for c in range(8):
                        nc.tensor.matmul(
                            xw[:, c, sl, :],
                            w1_sb[:, 2:4, c * 128:(c + 1) * 128],
                            h1T_tm1[:],
                            start=False, stop=True, perf_mode=DR)

                def emit_l2(t, h1T_tm1):
                    """L2(t-1) gates [128, 8, 32]; k-group 0 = h1(t-1),
                    k-group 1 = h2(t-2) (skipped at t==1 where h2 is zero)."""
                    psz2 = zp.tile([128, 8, 32], F32, tag="z2")
                    for c in bch2:
                        nc.tensor.matmul(
                            psz2[:, c, :],
                            b2x_sb[0:1, c * 128:(c + 1) * 128],
                            ones_row[0:1, 0:32], start=True, stop=False)
                    ng = 1 if t == 1 else 2
                    tq0 = 32 * (t - 2)
                    for c in range(8):
                        for kg in range(ng):
                            rhs = (h1T_tm1[:] if kg == 0
                                   else hs8[:, :, tq0:tq0 + 32])
                            nc.tensor.matmul(
                                psz2[:, c, :],
                                w2_sb[:, 2 * kg:2 * kg + 2,
                                      c * 128:(c + 1) * 128],
                                rhs,
                                start=(kg == 0 and c not in bch2),
                                stop=(kg == ng - 1), perf_mode=DR)
                    return psz2

                def lstm_act(psz):
                    """One tanh over the transposed gate chunks
                    [i,i,o,o,j,j,f,f].  W carries 8x the (0.5-sigmoid-folded)
                    weights to keep fp8 out of denormals; the ACT input
                    scale 0.125 undoes it.  f bias: +4.0 in psum -> +0.5."""
                    g = lw.tile([128, 8, 32], BF16, tag="g")
                    nc.scalar.activation(g[:], psz, AF.Tanh, scale=0.125)
                    return g

                def lstm_cell(g, c_sb, hout):
                    """Cell state kept as C' = 2c.  t1' = (g_i+1)*g_j
                    (= 2*sig(i)*tanh(j)); sig(f)*C' and the add on DVE;
                    tanh(c) via the ACT input scale (0.5 * C')."""
                    t1 = lw.tile([128, 2, 32], F32, tag="t1")
                    nc.vector.scalar_tensor_tensor(
                        out=t1[:], in0=g[:, 0:2, :], scalar=1.0,
                        in1=g[:, 4:6, :], op0=ALU.add, op1=ALU.mult)
                    junk = lw.tile([128, 1], F32, tag="junk")
                    cf = lw.tile([128, 2, 32], F32, tag="cf")
                    nc.vector.affine_mul_reduce(
                        cf[:], junk[:], g[:, 6:8, :], c_sb[:], 0.5, 0.5)
                    nc.vector.tensor_tensor(out=c_sb[:], in0=cf[:],
                                            in1=t1[:], op=ALU.add)
                    tc_t = lw.tile([128, 2, 32], BF16, tag="tc")
                    nc.scalar.activation(tc_t[:], c_sb[:], AF.Tanh,
                                         scale=0.5)
                    nc.vector.affine_mul_reduce(
                        hout, junk[:], g[:, 2:4, :], tc_t[:], 0.5, 0.5)

                # ---- wavefront: slot t = L1 step t  +  L2 step t-1 ----
                h1T_prev = h0T
                xw_cur = emit_xwindow(0)
                xw_next = None
                pending_exp = None
                for t in range(T + 1):
                    ei = t - 6
                    h1T_tm1 = h1T_prev

                    # previous slot's exp first: its psum is ready, so it
                    # fills the ACT engine while this slot's MMs run
                    if pending_exp is not None:
                        emit_exp(pending_exp)
                        pending_exp = None
                    # PE fill while the chain runs
                    if ei >= 0:
                        pending_exp = emit_logits_mms(ei // 4, ei % 4)
                    # L1(t) h-part (waits on h1T(t-1); zero h at t=0)
                    if t < T:
                        emit_l1_h(xw_cur, t, h1T_tm1)
                    # L2(t-1): all inputs ready at slot start
                    psz2 = emit_l2(t, h1T_tm1) if t >= 1 else None

                    # incremental embedding gather, 6 tiles ahead
                    if t % 4 == 0 and t // 4 + 6 < RT:
                        emit_gather(t // 4 + 6)

                    # both gate ACTs first (the second runs while the first
                    # layer's DVE cell chain executes), then the cell chains
                    g1 = lstm_act(xw_cur[:, :, t % 2, :]) if t < T else None
                    g2 = lstm_act(psz2[:]) if psz2 is not None else None
                    if g1 is not None:
                        h1T = lw.tile([128, 2, 32], F8, tag="h1T")
                        lstm_cell(g1, c1, h1T[:])
                        h1T_prev = h1T
                    if g2 is not None:
                        tp0 = 32 * (t - 1)
                        lstm_cell(g2, c2, hs8[:, :, tp0:tp0 + 32])
                        # bf16 copy of the same fp8 values for the
                        # target-logit path (exact upcast, off-chain)
                        nc.gpsimd.tensor_copy(hs[:, :, tp0:tp0 + 32],
                                              hs8[:, :, tp0:tp0 + 32])
                    # prefetch the next x window into the PE tail
                    if t % 2 == 0 and t + 2 < T:
                        xw_next = emit_xwindow(t // 2 + 1)
                    elif t % 2 == 1:
                        xw_cur = xw_next

                # trailing logits pairs
                for ei in range(T - 5, RT * NPAIR):
                    if pending_exp is not None:
                        emit_exp(pending_exp)
                    pending_exp = emit_logits_mms(ei // 4, ei % 4)
                emit_exp(pending_exp)

            nc.sync.dma_start(se_d[:], se_sb[:])
            nc.sync.dma_start(tg_d[:], tg_sb[:])

    nc.compile()
    meta = dict(T=T, V=V, n_cores=n_cores, B=B, H=H, VS=VS, BT=BT, RT=RT,
                CH=CH, NCHUNK=NCHUNK, NPAIR=NPAIR)
    return nc, meta


# ---------------- host-side prep / combine ----------------

def prep_inputs(meta, input_data, targets, embedding, W1, b1, W2, b2,
                softmax_w, softmax_b):
    """Build the per-core input maps (numpy)."""
    B, T, V = meta["B"], meta["T"], meta["V"]
    VS, RT, n_cores = meta["VS"], meta["RT"], meta["n_cores"]
    H = meta["H"]
    G4 = 4 * H

    ids_tm = np.ascontiguousarray(
        np.asarray(input_data, np.int64).T).reshape(-1)
    tgt_tm = np.ascontiguousarray(
        np.asarray(targets, np.int64).T).reshape(-1)
    ids_in = ids_tm.astype(np.int32).reshape(RT, 128, 1)

    # W column permutation [i, j, f, o] (TF order) -> [i, o, j, f], with the
    # 0.5 sigmoid input scale folded into the i/o/f columns (the device adds
    # +0.5 to the f columns in psum and does one plain tanh over all gates)
    perm = np.concatenate([
        np.arange(0, H), np.arange(3 * H, 4 * H),
        np.arange(H, 2 * H), np.arange(2 * H, 3 * H)])
    gate_scale = np.concatenate([
        np.full(2 * H, 0.5, np.float32),          # i, o
        np.ones(H, np.float32),                   # j
        np.full(H, 0.5, np.float32)])             # f

    def prep_w(W):
        # 8x the folded weights keeps fp8 e4m3 in its normal range (the
        # device undoes it with the ACT input scale 0.125)
        Wp = (W[:, perm] * (gate_scale * 8.0)[None, :]).astype(NPF8)
        return np.ascontiguousarray(Wp.reshape(4, 128, G4))

    w1_in = prep_w(np.asarray(W1, np.float32))
    w2_in = prep_w(np.asarray(W2, np.float32))
    # combined per-gate bias row (x8, pre-ACT-scale): perm+scaled b plus
    # the forget +0.5
    fhalf = np.zeros(G4, np.float32)
    fhalf[3 * H:] = 0.5
    b1x = (8.0 * ((np.asarray(b1, np.float32)[perm] * gate_scale) + fhalf)
           ).astype(ml_dtypes.bfloat16).reshape(1, G4)
    b2x = (8.0 * ((np.asarray(b2, np.float32)[perm] * gate_scale) + fhalf)
           ).astype(ml_dtypes.bfloat16).reshape(1, G4)

    sw = np.asarray(softmax_w, np.float32)                  # [H, V]
    swb = np.asarray(softmax_b, np.float32)

    # vectorized ap_gather index layout: idx i lives at partition i%16,
    # column i//16, replicated per 16-partition group
    rtA = (np.arange(RT) * 128)[:, None, None]
    pA = (np.arange(128) % 16)[None, :, None]
    qA = (np.arange(8) * 16)[None, None, :]
    gat = rtA + qA + pA                                     # [RT, 128, 8]

    maps, masks = [], []
    for c in range(n_cores):
        shard8 = sw[:, c * VS:(c + 1) * VS].astype(NPF8)
        sw_in = np.ascontiguousarray(shard8.reshape(2, 128, VS))
        # swp carries the SAME fp8-rounded values, upcast to bf16 (exact),
        # for the ap_gather target-logit path
        swb16 = shard8.astype(ml_dtypes.bfloat16).reshape(2, 128, VS)
        swi = swb16.view(np.int16)
        swp_in = np.ascontiguousarray(
            np.stack([swi, swi], axis=-1))                  # [2,128,VS,2]

        tl = tgt_tm - c * VS
        inr = (tl >= 0) & (tl < VS)
        tlc = np.where(inr, tl, 0).astype(np.int16)
        tgi = tlc[gat]                                      # [RT, 128, 8]
        m = dict(ids=ids_in, emb=np.asarray(embedding, np.float32),
                 w1=w1_in, w2=w2_in, sw=sw_in, swp=swp_in, tgi=tgi,
                 b1x=b1x, b2x=b2x)
        if np.any(swb):
            m["swbp"] = np.ascontiguousarray(
                np.tile(swb[c * VS:(c + 1) * VS].reshape(1, VS), (128, 1)))
        maps.append(m)
        masks.append(inr.astype(np.float32))
    return maps, masks, ids_tm, tgt_tm


def combine_outputs(meta, results, masks, tgt_tm, softmax_b):
    """results: list of per-core dicts with se_out [128, RT*NPAIR] and
    tg_out [1, BT]. Returns the scalar cost (np.float32)."""
    B, T, BT = meta["B"], meta["T"], meta["BT"]
    RT, NPAIR = meta["RT"], meta["NPAIR"]
    se_all = np.zeros(BT, np.float64)
    tg_all = np.zeros(BT, np.float64)
    for c, r in enumerate(results):
        se = np.asarray(r["se_out"], np.float64)  # [128, RT*NPAIR]
        se = se.reshape(128, RT, NPAIR).sum(-1)   # [128, RT]
        se_all += se.T.reshape(-1)                # row r = rt*128 + p
        tg_all += np.asarray(r["tg_out"], np.float64)[0] * masks[c]
    tg_all += np.asarray(softmax_b, np.float64)[tgt_tm]
    loss = np.log(se_all) - tg_all
    return np.float32(loss.sum() / B / T)


# ---------------- public entry point ----------------

_CACHE = {}
last_exec_time_ns = None
last_trace_path = None


def _get_built(has_b1, has_b2, has_swb):
    key = (has_b1, has_b2, has_swb)
    if key not in _CACHE:
        _CACHE[key] = build_charrnn(T=T, V=V, n_cores=NCORES,
                                    has_b1=has_b1, has_b2=has_b2,
                                    has_swb=has_swb, num_devices=NCORES)
    return _CACHE[key]


def kernel(input_data, targets, embedding, W1, b1, W2, b2,
           softmax_w, softmax_b, _trace=False):
    global last_exec_time_ns, last_trace_path
    has_b1 = bool(np.any(np.asarray(b1)))
    has_b2 = bool(np.any(np.asarray(b2)))
    has_swb = bool(np.any(np.asarray(softmax_b)))
    nc, meta = _get_built(has_b1, has_b2, has_swb)
    maps, masks, ids_tm, tgt_tm = prep_inputs(
        meta, input_data, targets, embedding, W1, b1, W2, b2,
        softmax_w, softmax_b)
    res = run_bass_kernel_spmd(nc, maps, core_ids=list(range(NCORES)),
                               trace=_trace)
    last_exec_time_ns = res.exec_time_ns
    if res.instructions_and_trace is not None:
        last_trace_path = res.instructions_and_trace[1]
    cost = combine_outputs(meta, res.results, masks, tgt_tm, softmax_b)
    return np.asarray(cost, np.float32)



# revision 22
# speedup vs baseline: 1.3702x; 1.3702x over previous
"""Self-contained Trainium2 Bass kernel for the CharRNN problem:
2-layer LSTM (B=32, T=256, H=256) + V=32000 softmax cross-entropy mean loss.

Strategy (8 NeuronCores, SPMD):
  * the LSTM recurrence is replicated on every core (latency-bound)
  * the softmax matmul + exp is sharded over the vocab: each core owns a
    4000-wide shard of softmax_w, computes logits for all 8192 rows against
    its shard, reduces them to per-row sum(exp(logit)) plus the per-row
    target logit; the host combines loss_r = log(sum_c se_r) - tgt_logit_r

Device-side structure (v4 — transposed "zT" gate layout):
  * wavefront: slot t runs L1 step t and L2 step t-1 so the two layer
    recurrence chains interleave on the engines
  * gates are computed TRANSPOSED: z^T chunks [128(gate), 32(batch)] via
    W-stationary matmuls (lhsT = W 128x128 chunk, rhs = hidden-major
    h/x k-tiles).  h comes OUT of the cell ops already hidden-major
    [128, 2, 32] -> no transposes at all, and every elementwise/ACT op
    runs on all 128 partitions (4x lane utilization vs the [32, *] form)
  * the x-part of L1's gates is batched over a 2-step window into the
    same psum the per-step h-part accumulates into (saves ~2K PE
    rows/step of W streaming)
  * cell state kept as C' = 2c: Pool computes t1' = (g_i+1)*g_j, DVE
    computes sig(f)*C' and the add, ACT computes tanh(C' * 0.5) using
    the activation input scale; gate col order [i, o, j, f] with the 0.5
    sigmoid input scale folded into W on the host; per-partition biases
    (forget +0.5) are added by K=1 matmuls (lhsT = bias row, rhs = ones)
  * exp over PAIRS of 500-wide vocab chunks ([128,2,500] strided AP);
    per slot the previous slot's exp is emitted FIRST on ACT (its psum is
    ready at slot start), then the logits MMs (PE fill), then the gates
"""
import numpy as np
import ml_dtypes
import concourse.bass as bass
import concourse.mybir as mybir
import concourse.tile as tile
from concourse import bacc
from concourse.bass_utils import run_bass_kernel_spmd

F32 = mybir.dt.float32
BF16 = mybir.dt.bfloat16
F8 = mybir.dt.float8e4
I32 = mybir.dt.int32
I16 = mybir.dt.int16
AF = mybir.ActivationFunctionType
ALU = mybir.AluOpType
DR = mybir.MatmulPerfMode.DoubleRow
NPF8 = ml_dtypes.float8_e4m3fn

B, T, H, V, NCORES = 32, 256, 256, 32000, 8


def build_charrnn(T=256, V=32000, n_cores=8, has_b1=False, has_b2=False,
                  has_swb=False, num_devices=8):
    B, H = 32, 256
    G4 = 4 * H                      # 1024 gate width
    VS = V // n_cores               # vocab shard per core
    BT = B * T
    RT = BT // 128                  # 128-row tiles (4 steps each)
    assert T % 4 == 0 and BT % 128 == 0

    # one psum BANK per matmul chunk (a matmul may not cross a bank)
    CH = max(d for d in range(1, 513) if VS % d == 0)   # 500
    NCHUNK = VS // CH                                    # 8
    NPAIR = NCHUNK // 2                                  # 4 exp calls per tile

    nc = bacc.Bacc("TRN2", target_bir_lowering=False, debug=False,
                   num_devices=num_devices)

    # ---------------- DRAM I/O ----------------
    ids_d = nc.dram_tensor("ids", (RT, 128, 1), I32, kind="ExternalInput")
    emb_d = nc.dram_tensor("emb", (V, H), F32, kind="ExternalInput")
    w1_d = nc.dram_tensor("w1", (4, 128, G4), BF16, kind="ExternalInput")
    w2_d = nc.dram_tensor("w2", (4, 128, G4), BF16, kind="ExternalInput")
    sw_d = nc.dram_tensor("sw", (2, 128, VS), F8, kind="ExternalInput")
    swp_d = nc.dram_tensor("swp", (2, 128, VS, 2), I16, kind="ExternalInput")
    tgi_d = nc.dram_tensor("tgi", (RT, 128, 8), I16, kind="ExternalInput")
    # combined per-gate bias rows (perm+scaled b + forget +0.5), always fed
    b1x_d = nc.dram_tensor("b1x", (1, G4), BF16, kind="ExternalInput")
    b2x_d = nc.dram_tensor("b2x", (1, G4), BF16, kind="ExternalInput")
    if has_swb:
        swb_d = nc.dram_tensor("swbp", (128, VS), F32, kind="ExternalInput")
    se_d = nc.dram_tensor("se_out", (128, RT * NPAIR), F32,
                          kind="ExternalOutput")
    tg_d = nc.dram_tensor("tg_out", (1, BT), F32, kind="ExternalOutput")

    with tile.TileContext(nc) as tc:
        with tc.tile_pool(name="persist", bufs=1) as pp:
            # ---- persistent SBUF ----
            w1_sb = pp.tile([128, 4, G4], BF16, tag="w1")
            w2_sb = pp.tile([128, 4, G4], BF16, tag="w2")
            nc.sync.dma_start(w1_sb[:], w1_d[:].rearrange("k p c -> p k c"))
            nc.sync.dma_start(w2_sb[:], w2_d[:].rearrange("k p c -> p k c"))
            sw_sb = pp.tile([128, 2, VS], F8, tag="sw")
            nc.sync.dma_start(sw_sb[:], sw_d[:].rearrange("k p c -> p k c"))
            swp_sb = pp.tile([128, 2, VS, 2], I16, tag="swp")
            nc.sync.dma_start(swp_sb[:],
                              swp_d[:].rearrange("k p c d -> p k c d"))
            hs = pp.tile([128, 2, BT], BF16, tag="hs")
            # fp8 copy of h2 for the DoubleRow logits matmuls
            hs8 = pp.tile([128, 2, BT], F8, tag="hs8")

            ones_bf = pp.tile([128, 1], BF16, tag="ones")
            nc.gpsimd.memset(ones_bf[:], 1.0)
            # ones row for K=1 per-partition bias matmuls (rhs side)
            ones_row = pp.tile([1, 64], BF16, tag="onesrow")
            nc.gpsimd.memset(ones_row[:], 1.0)
            # per-gate bias rows (lhsT side of the K=1 bias matmuls)
            b1x_sb = pp.tile([1, G4], BF16, tag="b1x")
            b2x_sb = pp.tile([1, G4], BF16, tag="b2x")
            nc.sync.dma_start(b1x_sb[:], b1x_d[:])
            nc.sync.dma_start(b2x_sb[:], b2x_d[:])
            # bias chunks: all 8 if a real b was provided, else just the
            # forget-gate chunks 6,7 (+0.5)
            bch1 = range(8) if has_b1 else range(6, 8)
            bch2 = range(8) if has_b2 else range(6, 8)

            # cell state, kept as C' = 2c, hidden-major [128, 2, 32]
            c1 = pp.tile([128, 2, 32], F32, tag="c1")
            c2 = pp.tile([128, 2, 32], F32, tag="c2")
            nc.gpsimd.memset(c1[:], 0.0)
            nc.gpsimd.memset(c2[:], 0.0)
            # zero h for the step-0 h-part matmuls
            h0T = pp.tile([128, 2, 32], BF16, tag="h0T")
            nc.gpsimd.memset(h0T[:], 0.0)

            se_sb = pp.tile([128, RT * NPAIR], F32, tag="se")
            tg_sb = pp.tile([1, BT], F32, tag="tg")
            # accum_out adds into existing SBUF content on HW — zero it
            nc.gpsimd.memset(se_sb[:], 0.0)

            if has_swb:
                swb_sb = pp.tile([128, VS], F32, tag="swb")
                nc.sync.dma_start(swb_sb[:], swb_d[:])

            # ============ fused phase: gather + LSTM + logits ============
            with (
                tc.tile_pool(name="xsp", bufs=1) as xsp,
                tc.tile_pool(name="stage", bufs=8) as stp,
                tc.tile_pool(name="lwork", bufs=3) as lw,
                tc.tile_pool(name="xwp", bufs=2, space="PSUM") as xwp,
                tc.tile_pool(name="zp", bufs=2, space="PSUM") as zp,
                tc.tile_pool(name="ep", bufs=2, space="PSUM") as ep,
                tc.tile_pool(name="ework", bufs=3) as ew,
            ):
                xs = xsp.tile([128, 2, BT], BF16, tag="xs")

                # ---- embedding gather (time-major) + transpose to slabs;
                # emitted incrementally from the slot loop so the engine
                # queues are ordered to match data arrival ----
                def emit_gather(rt):
                    ids_sb = stp.tile([128, 1], I32, tag="ids")
                    nc.gpsimd.dma_start(ids_sb[:], ids_d.ap()[rt])
                    xrow = stp.tile([128, H], F32, tag="xrow")
                    nc.gpsimd.indirect_dma_start(
                        out=xrow[:], out_offset=None,
                        in_=emb_d[:],
                        in_offset=bass.IndirectOffsetOnAxis(
                            ap=ids_sb[:, :1], axis=0),
                    )
                    xbf = stp.tile([128, H], BF16, tag="xbf")
                    nc.gpsimd.tensor_copy(xbf[:], xrow[:])
                    cs = 128 * rt
                    nc.sync.dma_start_transpose(
                        xs[:, 0, cs:cs + 128], xbf[:, 0:128])
                    nc.sync.dma_start_transpose(
                        xs[:, 1, cs:cs + 128], xbf[:, 128:256])

                for rt in range(6):
                    emit_gather(rt)

                def emit_logits_mms(rt, p):
                    """Logits matmuls for vocab chunks (2p, 2p+1) of row-tile
                    rt; p==3 also emits the target-logit gather+reduce.
                    Returns state for the deferred exp/copy emission."""
                    cs = 128 * rt
                    pse = ep.tile([128, 2, 512], F32, tag="pse")
                    for half, c in enumerate((2 * p, 2 * p + 1)):
                        nc.tensor.matmul(
                            pse[:, half, 0:CH], hs8[:, :, cs:cs + 128],
                            sw_sb[:, :, c * CH:c * CH + CH],
                            start=True, stop=True, perf_mode=DR,
                        )
                        if has_swb:
                            nc.vector.tensor_tensor(
                                out=pse[:, half, 0:CH], in0=pse[:, half, 0:CH],
                                in1=swb_sb[:, (2 * p + half) * CH:
                                           (2 * p + half) * CH + CH],
                                op=ALU.add)
                    pst = None
                    if p == 3:
                        tgi_sb = ew.tile([128, 8], I16, tag="tgi")
                        nc.gpsimd.dma_start(tgi_sb[:], tgi_d.ap()[rt])
                        pstt = ep.tile([128, 2, 512], F32, tag="pse")
                        pst = pstt[0:1, 0, 0:128]
                        for k in range(2):
                            swg = ew.tile([128, 128, 2], I16, tag="swg")
                            nc.gpsimd.ap_gather(
                                swg[:], swp_sb[:, k], tgi_sb[:],
                                channels=128, num_elems=VS, d=2, num_idxs=128,
                            )
                            mulk = ew.tile([128, 128], BF16, tag="mulk")
                            nc.vector.tensor_tensor(
                                out=mulk[:],
                                in0=swg[:].bitcast(BF16)[:, :, 0],
                                in1=hs[:, k, cs:cs + 128],
                                op=ALU.mult)
                            nc.tensor.matmul(pst, ones_bf[:, 0:1], mulk[:],
                                             start=(k == 0), stop=(k == 1))
                    return pse, pst, rt, p, cs

                def emit_exp(state):
                    pse, pst, rt, p, cs = state
                    ebuf = ew.tile([128, 2, CH], BF16, tag="ebuf")
                    nc.scalar.activation(
                        ebuf[:], pse[:, :, 0:CH], AF.Exp,
                        accum_out=se_sb[:, rt * NPAIR + p:rt * NPAIR + p + 1])
                    if pst is not None:
                        nc.scalar.copy(tg_sb[0:1, cs:cs + 128], pst)

                def emit_xwindow(m):
                    """L1 x-part + bias MMs for steps (2m, 2m+1) into a
                    fresh [128, 8chunk, 2step, 32batch] psum (one bank).
                    The bias matmuls OPEN their chunks' accumulation; the
                    per-step h-part closes each step's column slice."""
                    t0 = 2 * m
                    xw = xwp.tile([128, 8, 2, 32], F32, tag="xw")
                    for c in bch1:
                        nc.tensor.matmul(
                            xw[:, c, :, :],
                            b1x_sb[0:1, c * 128:(c + 1) * 128],
                            ones_row[0:1, 0:64], start=True, stop=False)
                    for c in range(8):
                        for kt in range(2):
                            nc.tensor.matmul(
                                xw[:, c, :, :],
                                w1_sb[:, kt, c * 128:(c + 1) * 128],
                                xs[:, kt, 32 * t0:32 * t0 + 64],
                                start=(kt == 0 and c not in bch1),
                                stop=False)
                    return xw

                def emit_l1_h(xw, t, h1T_tm1):
                    """L1(t) h-part: W-stationary matmuls into this step's
                    column slice of the x-window psum."""
                    sl = t % 2
                    for c in range(8):
                        for kt in range(2):
                            nc.tensor.matmul(
                                xw[:, c, sl, :],
                                w1_sb[:, 2 + kt, c * 128:(c + 1) * 128],
                                h1T_tm1[:, kt, :],
                                start=False, stop=(kt == 1))

                def emit_l2(t, h1T_tm1):
                    """L2(t-1) gates [128, 8, 32]; kt 0,1 = h1(t-1),
                    kt 2,3 = h2(t-2) (skipped at t==1 where h2 is zero)."""
                    psz2 = zp.tile([128, 8, 32], F32, tag="z2")
                    for c in bch2:
                        nc.tensor.matmul(
                            psz2[:, c, :],
                            b2x_sb[0:1, c * 128:(c + 1) * 128],
                            ones_row[0:1, 0:32], start=True, stop=False)
                    nkt = 2 if t == 1 else 4
                    tq0 = 32 * (t - 2)
                    for c in range(8):
                        for kt in range(nkt):
                            rhs = (h1T_tm1[:, kt, :] if kt < 2
                                   else hs[:, kt - 2, tq0:tq0 + 32])
                            nc.tensor.matmul(
                                psz2[:, c, :],
                                w2_sb[:, kt, c * 128:(c + 1) * 128],
                                rhs,
                                start=(kt == 0 and c not in bch2),
                                stop=(kt == nkt - 1))
                    return psz2

                def lstm_act(psz):
                    """One plain tanh over the transposed gate chunks
                    [i,i,o,o,j,j,f,f] (sigmoid input scales pre-folded into
                    W; f +0.5 bias pre-added in psum)."""
                    g = lw.tile([128, 8, 32], BF16, tag="g")
                    nc.scalar.activation(g[:], psz, AF.Tanh)
                    return g

                def lstm_cell(g, c_sb, hout):
                    """Cell state kept as C' = 2c.  t1' = (g_i+1)*g_j
                    (= 2*sig(i)*tanh(j)); sig(f)*C' and the add on DVE;
                    tanh(c) via the ACT input scale (0.5 * C')."""
                    t1 = lw.tile([128, 2, 32], F32, tag="t1")
                    nc.vector.scalar_tensor_tensor(
                        out=t1[:], in0=g[:, 0:2, :], scalar=1.0,
                        in1=g[:, 4:6, :], op0=ALU.add, op1=ALU.mult)
                    junk = lw.tile([128, 1], F32, tag="junk")
                    cf = lw.tile([128, 2, 32], F32, tag="cf")
                    nc.vector.affine_mul_reduce(
                        cf[:], junk[:], g[:, 6:8, :], c_sb[:], 0.5, 0.5)
                    nc.vector.tensor_tensor(out=c_sb[:], in0=cf[:],
                                            in1=t1[:], op=ALU.add)
                    tc_t = lw.tile([128, 2, 32], BF16, tag="tc")
                    nc.scalar.activation(tc_t[:], c_sb[:], AF.Tanh,
                                         scale=0.5)
                    nc.vector.affine_mul_reduce(
                        hout, junk[:], g[:, 2:4, :], tc_t[:], 0.5, 0.5)

                # ---- wavefront: slot t = L1 step t  +  L2 step t-1 ----
                h1T_prev = h0T
                xw_cur = emit_xwindow(0)
                xw_next = None
                pending_exp = None
                for t in range(T + 1):
                    ei = t - 6
                    h1T_tm1 = h1T_prev

                    # previous slot's exp first: its psum is ready, so it
                    # fills the ACT engine while this slot's MMs run
                    if pending_exp is not None:
                        emit_exp(pending_exp)
                        pending_exp = None
                    # PE fill while the chain runs
                    if ei >= 0:
                        pending_exp = emit_logits_mms(ei // 4, ei % 4)
                    # L1(t) h-part (waits on h1T(t-1); zero h at t=0)
                    if t < T:
                        emit_l1_h(xw_cur, t, h1T_tm1)
                    # L2(t-1): all inputs ready at slot start
                    psz2 = emit_l2(t, h1T_tm1) if t >= 1 else None

                    # incremental embedding gather, 6 tiles ahead
                    if t % 4 == 0 and t // 4 + 6 < RT:
                        emit_gather(t // 4 + 6)
                    # fp8 copy of each completed 128-col h2 tile (feeds the
                    # DoubleRow logits matmuls, emitted >=1 slot later)
                    if t >= 5 and t % 4 == 1 and (t - 5) // 4 < RT - 1:
                        cd = 128 * ((t - 5) // 4)
                        nc.gpsimd.tensor_copy(hs8[:, :, cd:cd + 128],
                                              hs[:, :, cd:cd + 128])

                    # both gate ACTs first (the second runs while the first
                    # layer's DVE cell chain executes), then the cell chains
                    g1 = lstm_act(xw_cur[:, :, t % 2, :]) if t < T else None
                    g2 = lstm_act(psz2[:]) if psz2 is not None else None
                    if g1 is not None:
                        h1T = lw.tile([128, 2, 32], BF16, tag="h1T")
                        lstm_cell(g1, c1, h1T[:])
                        h1T_prev = h1T
                    if g2 is not None:
                        tp0 = 32 * (t - 1)
                        lstm_cell(g2, c2, hs[:, :, tp0:tp0 + 32])
                    # prefetch the next x window into the PE tail
                    if t % 2 == 0 and t + 2 < T:
                        xw_next = emit_xwindow(t // 2 + 1)
                    elif t % 2 == 1:
                        xw_cur = xw_next

                # last h2 tile fp8 copy, then trailing logits pairs
                cd = 128 * (RT - 1)
                nc.gpsimd.tensor_copy(hs8[:, :, cd:cd + 128],
                                      hs[:, :, cd:cd + 128])
                for ei in range(T - 5, RT * NPAIR):
                    if pending_exp is not None:
                        emit_exp(pending_exp)
                    pending_exp = emit_logits_mms(ei // 4, ei % 4)
                emit_exp(pending_exp)

            nc.sync.dma_start(se_d[:], se_sb[:])
            nc.sync.dma_start(tg_d[:], tg_sb[:])

    nc.compile()
    meta = dict(T=T, V=V, n_cores=n_cores, B=B, H=H, VS=VS, BT=BT, RT=RT,
                CH=CH, NCHUNK=NCHUNK, NPAIR=NPAIR)
    return nc, meta


# ---------------- host-side prep / combine ----------------

def prep_inputs(meta, input_data, targets, embedding, W1, b1, W2, b2,
                softmax_w, softmax_b):
    """Build the per-core input maps (numpy)."""
    B, T, V = meta["B"], meta["T"], meta["V"]
    VS, RT, n_cores = meta["VS"], meta["RT"], meta["n_cores"]
    H = meta["H"]
    G4 = 4 * H

    ids_tm = np.ascontiguousarray(
        np.asarray(input_data, np.int64).T).reshape(-1)
    tgt_tm = np.ascontiguousarray(
        np.asarray(targets, np.int64).T).reshape(-1)
    ids_in = ids_tm.astype(np.int32).reshape(RT, 128, 1)

    # W column permutation [i, j, f, o] (TF order) -> [i, o, j, f], with the
    # 0.5 sigmoid input scale folded into the i/o/f columns (the device adds
    # +0.5 to the f columns in psum and does one plain tanh over all gates)
    perm = np.concatenate([
        np.arange(0, H), np.arange(3 * H, 4 * H),
        np.arange(H, 2 * H), np.arange(2 * H, 3 * H)])
    gate_scale = np.concatenate([
        np.full(2 * H, 0.5, np.float32),          # i, o
        np.ones(H, np.float32),                   # j
        np.full(H, 0.5, np.float32)])             # f

    def prep_w(W):
        Wp = (W[:, perm] * gate_scale[None, :]).astype(ml_dtypes.bfloat16)
        return np.ascontiguousarray(Wp.reshape(4, 128, G4))

    w1_in = prep_w(np.asarray(W1, np.float32))
    w2_in = prep_w(np.asarray(W2, np.float32))
    # combined per-gate bias row: perm+scaled b plus the forget +0.5
    fhalf = np.zeros(G4, np.float32)
    fhalf[3 * H:] = 0.5
    b1x = ((np.asarray(b1, np.float32)[perm] * gate_scale) + fhalf
           ).astype(ml_dtypes.bfloat16).reshape(1, G4)
    b2x = ((np.asarray(b2, np.float32)[perm] * gate_scale) + fhalf
           ).astype(ml_dtypes.bfloat16).reshape(1, G4)

    sw = np.asarray(softmax_w, np.float32)                  # [H, V]
    swb = np.asarray(softmax_b, np.float32)

    # vectorized ap_gather index layout: idx i lives at partition i%16,
    # column i//16, replicated per 16-partition group
    rtA = (np.arange(RT) * 128)[:, None, None]
    pA = (np.arange(128) % 16)[None, :, None]
    qA = (np.arange(8) * 16)[None, None, :]
    gat = rtA + qA + pA                                     # [RT, 128, 8]

    maps, masks = [], []
    for c in range(n_cores):
        shard8 = sw[:, c * VS:(c + 1) * VS].astype(NPF8)
        sw_in = np.ascontiguousarray(shard8.reshape(2, 128, VS))
        # swp carries the same fp8-rounded values upcast to bf16 (exact)
        swb16 = shard8.astype(ml_dtypes.bfloat16).reshape(2, 128, VS)
        swi = swb16.view(np.int16)
        swp_in = np.ascontiguousarray(
            np.stack([swi, swi], axis=-1))                  # [2,128,VS,2]

        tl = tgt_tm - c * VS
        inr = (tl >= 0) & (tl < VS)
        tlc = np.where(inr, tl, 0).astype(np.int16)
        tgi = tlc[gat]                                      # [RT, 128, 8]
        m = dict(ids=ids_in, emb=np.asarray(embedding, np.float32),
                 w1=w1_in, w2=w2_in, sw=sw_in, swp=swp_in, tgi=tgi,
                 b1x=b1x, b2x=b2x)
        if np.any(swb):
            m["swbp"] = np.ascontiguousarray(
                np.tile(swb[c * VS:(c + 1) * VS].reshape(1, VS), (128, 1)))
        maps.append(m)
        masks.append(inr.astype(np.float32))
    return maps, masks, ids_tm, tgt_tm


def combine_outputs(meta, results, masks, tgt_tm, softmax_b):
    """results: list of per-core dicts with se_out [128, RT*NPAIR] and
    tg_out [1, BT]. Returns the scalar cost (np.float32)."""
    B, T, BT = meta["B"], meta["T"], meta["BT"]
    RT, NPAIR = meta["RT"], meta["NPAIR"]
    se_all = np.zeros(BT, np.float64)
    tg_all = np.zeros(BT, np.float64)
    for c, r in enumerate(results):
        se = np.asarray(r["se_out"], np.float64)  # [128, RT*NPAIR]
        se = se.reshape(128, RT, NPAIR).sum(-1)   # [128, RT]
        se_all += se.T.reshape(-1)                # row r = rt*128 + p
        tg_all += np.asarray(r["tg_out"], np.float64)[0] * masks[c]
    tg_all += np.asarray(softmax_b, np.float64)[tgt_tm]
    loss = np.log(se_all) - tg_all
    return np.float32(loss.sum() / B / T)


# ---------------- public entry point ----------------

_CACHE = {}
last_exec_time_ns = None
last_trace_path = None


def _get_built(has_b1, has_b2, has_swb):
    key = (has_b1, has_b2, has_swb)
    if key not in _CACHE:
        _CACHE[key] = build_charrnn(T=T, V=V, n_cores=NCORES,
                                    has_b1=has_b1, has_b2=has_b2,
                                    has_swb=has_swb, num_devices=NCORES)
    return _CACHE[key]


def kernel(input_data, targets, embedding, W1, b1, W2, b2,
           softmax_w, softmax_b, _trace=False):
    global last_exec_time_ns, last_trace_path
    has_b1 = bool(np.any(np.asarray(b1)))
    has_b2 = bool(np.any(np.asarray(b2)))
    has_swb = bool(np.any(np.asarray(softmax_b)))
    nc, meta = _get_built(has_b1, has_b2, has_swb)
    maps, masks, ids_tm, tgt_tm = prep_inputs(
        meta, input_data, targets, embedding, W1, b1, W2, b2,
        softmax_w, softmax_b)
    res = run_bass_kernel_spmd(nc, maps, core_ids=list(range(NCORES)),
                               trace=_trace)
    last_exec_time_ns = res.exec_time_ns
    if res.instructions_and_trace is not None:
        last_trace_path = res.instructions_and_trace[1]
    cost = combine_outputs(meta, res.results, masks, tgt_tm, softmax_b)
    return np.asarray(cost, np.float32)



# revision 23
# speedup vs baseline: 1.3838x; 1.0099x over previous
"""Self-contained Trainium2 Bass kernel for the CharRNN problem:
2-layer LSTM (B=32, T=256, H=256) + V=32000 softmax cross-entropy mean loss.

Strategy (8 NeuronCores, SPMD):
  * the LSTM recurrence is replicated on every core (latency-bound)
  * the softmax matmul + exp is sharded over the vocab: each core owns a
    4000-wide shard of softmax_w, computes logits for all 8192 rows against
    its shard, reduces them to per-row sum(exp(logit)) plus the per-row
    target logit; the host combines loss_r = log(sum_c se_r) - tgt_logit_r

Device-side structure (v4 — transposed "zT" gate layout):
  * wavefront: slot t runs L1 step t and L2 step t-1 so the two layer
    recurrence chains interleave on the engines
  * gates are computed TRANSPOSED: z^T chunks [128(gate), 32(batch)] via
    W-stationary matmuls (lhsT = W 128x128 chunk, rhs = hidden-major
    h/x k-tiles).  h comes OUT of the cell ops already hidden-major
    [128, 2, 32] -> no transposes at all, and every elementwise/ACT op
    runs on all 128 partitions (4x lane utilization vs the [32, *] form)
  * the x-part of L1's gates is batched over a 2-step window into the
    same psum the per-step h-part accumulates into (saves ~2K PE
    rows/step of W streaming)
  * cell state kept as C' = 2c: Pool computes t1' = (g_i+1)*g_j, DVE
    computes sig(f)*C' and the add, ACT computes tanh(C' * 0.5) using
    the activation input scale; gate col order [i, o, j, f] with the 0.5
    sigmoid input scale folded into W on the host; per-partition biases
    (forget +0.5) are added by K=1 matmuls (lhsT = bias row, rhs = ones)
  * exp over PAIRS of 500-wide vocab chunks ([128,2,500] strided AP);
    per slot the previous slot's exp is emitted FIRST on ACT (its psum is
    ready at slot start), then the logits MMs (PE fill), then the gates
"""
import numpy as np
import ml_dtypes
import concourse.bass as bass
import concourse.mybir as mybir
import concourse.tile as tile
from concourse import bacc
from concourse.bass_utils import run_bass_kernel_spmd

F32 = mybir.dt.float32
BF16 = mybir.dt.bfloat16
I32 = mybir.dt.int32
I16 = mybir.dt.int16
AF = mybir.ActivationFunctionType
ALU = mybir.AluOpType

B, T, H, V, NCORES = 32, 256, 256, 32000, 8


def build_charrnn(T=256, V=32000, n_cores=8, has_b1=False, has_b2=False,
                  has_swb=False, num_devices=8):
    B, H = 32, 256
    G4 = 4 * H                      # 1024 gate width
    VS = V // n_cores               # vocab shard per core
    BT = B * T
    RT = BT // 128                  # 128-row tiles (4 steps each)
    assert T % 4 == 0 and BT % 128 == 0

    # one psum BANK per matmul chunk (a matmul may not cross a bank)
    CH = max(d for d in range(1, 513) if VS % d == 0)   # 500
    NCHUNK = VS // CH                                    # 8
    NPAIR = NCHUNK // 2                                  # 4 exp calls per tile

    nc = bacc.Bacc("TRN2", target_bir_lowering=False, debug=False,
                   num_devices=num_devices)

    # ---------------- DRAM I/O ----------------
    ids_d = nc.dram_tensor("ids", (RT, 128, 1), I32, kind="ExternalInput")
    emb_d = nc.dram_tensor("emb", (V, H), F32, kind="ExternalInput")
    w1_d = nc.dram_tensor("w1", (4, 128, G4), BF16, kind="ExternalInput")
    w2_d = nc.dram_tensor("w2", (4, 128, G4), BF16, kind="ExternalInput")
    sw_d = nc.dram_tensor("sw", (2, 128, VS), BF16, kind="ExternalInput")
    swp_d = nc.dram_tensor("swp", (2, 128, VS, 2), I16, kind="ExternalInput")
    tgi_d = nc.dram_tensor("tgi", (RT, 128, 8), I16, kind="ExternalInput")
    # combined per-gate bias rows (perm+scaled b + forget +0.5), always fed
    b1x_d = nc.dram_tensor("b1x", (1, G4), BF16, kind="ExternalInput")
    b2x_d = nc.dram_tensor("b2x", (1, G4), BF16, kind="ExternalInput")
    if has_swb:
        swb_d = nc.dram_tensor("swbp", (128, VS), F32, kind="ExternalInput")
    se_d = nc.dram_tensor("se_out", (128, RT * NPAIR), F32,
                          kind="ExternalOutput")
    tg_d = nc.dram_tensor("tg_out", (1, BT), F32, kind="ExternalOutput")

    with tile.TileContext(nc) as tc:
        with tc.tile_pool(name="persist", bufs=1) as pp:
            # ---- persistent SBUF ----
            w1_sb = pp.tile([128, 4, G4], BF16, tag="w1")
            w2_sb = pp.tile([128, 4, G4], BF16, tag="w2")
            nc.sync.dma_start(w1_sb[:], w1_d[:].rearrange("k p c -> p k c"))
            nc.sync.dma_start(w2_sb[:], w2_d[:].rearrange("k p c -> p k c"))
            sw_sb = pp.tile([128, 2, VS], BF16, tag="sw")
            nc.sync.dma_start(sw_sb[:], sw_d[:].rearrange("k p c -> p k c"))
            swp_sb = pp.tile([128, 2, VS, 2], I16, tag="swp")
            nc.sync.dma_start(swp_sb[:],
                              swp_d[:].rearrange("k p c d -> p k c d"))
            hs = pp.tile([128, 2, BT], BF16, tag="hs")

            ones_bf = pp.tile([128, 1], BF16, tag="ones")
            nc.gpsimd.memset(ones_bf[:], 1.0)
            # ones row for K=1 per-partition bias matmuls (rhs side)
            ones_row = pp.tile([1, 64], BF16, tag="onesrow")
            nc.gpsimd.memset(ones_row[:], 1.0)
            # per-gate bias rows (lhsT side of the K=1 bias matmuls)
            b1x_sb = pp.tile([1, G4], BF16, tag="b1x")
            b2x_sb = pp.tile([1, G4], BF16, tag="b2x")
            nc.sync.dma_start(b1x_sb[:], b1x_d[:])
            nc.sync.dma_start(b2x_sb[:], b2x_d[:])
            # bias chunks: all 8 if a real b was provided, else just the
            # forget-gate chunks 6,7 (+0.5)
            bch1 = range(8) if has_b1 else range(6, 8)
            bch2 = range(8) if has_b2 else range(6, 8)

            # cell state, kept as C' = 2c, hidden-major [128, 2, 32]
            c1 = pp.tile([128, 2, 32], F32, tag="c1")
            c2 = pp.tile([128, 2, 32], F32, tag="c2")
            nc.gpsimd.memset(c1[:], 0.0)
            nc.gpsimd.memset(c2[:], 0.0)
            # zero h for the step-0 h-part matmuls
            h0T = pp.tile([128, 2, 32], BF16, tag="h0T")
            nc.gpsimd.memset(h0T[:], 0.0)

            se_sb = pp.tile([128, RT * NPAIR], F32, tag="se")
            tg_sb = pp.tile([1, BT], F32, tag="tg")
            # accum_out adds into existing SBUF content on HW — zero it
            nc.gpsimd.memset(se_sb[:], 0.0)

            if has_swb:
                swb_sb = pp.tile([128, VS], F32, tag="swb")
                nc.sync.dma_start(swb_sb[:], swb_d[:])

            # ============ fused phase: gather + LSTM + logits ============
            with (
                tc.tile_pool(name="xsp", bufs=1) as xsp,
                tc.tile_pool(name="stage", bufs=8) as stp,
                tc.tile_pool(name="lwork", bufs=3) as lw,
                tc.tile_pool(name="xwp", bufs=2, space="PSUM") as xwp,
                tc.tile_pool(name="zp", bufs=2, space="PSUM") as zp,
                tc.tile_pool(name="ep", bufs=2, space="PSUM") as ep,
                tc.tile_pool(name="ework", bufs=3) as ew,
            ):
                xs = xsp.tile([128, 2, BT], BF16, tag="xs")

                # ---- embedding gather (time-major) + transpose to slabs;
                # emitted incrementally from the slot loop so the engine
                # queues are ordered to match data arrival ----
                def emit_gather(rt):
                    ids_sb = stp.tile([128, 1], I32, tag="ids")
                    nc.gpsimd.dma_start(ids_sb[:], ids_d.ap()[rt])
                    xrow = stp.tile([128, H], F32, tag="xrow")
                    nc.gpsimd.indirect_dma_start(
                        out=xrow[:], out_offset=None,
                        in_=emb_d[:],
                        in_offset=bass.IndirectOffsetOnAxis(
                            ap=ids_sb[:, :1], axis=0),
                    )
                    xbf = stp.tile([128, H], BF16, tag="xbf")
                    nc.gpsimd.tensor_copy(xbf[:], xrow[:])
                    cs = 128 * rt
                    nc.sync.dma_start_transpose(
                        xs[:, 0, cs:cs + 128], xbf[:, 0:128])
                    nc.sync.dma_start_transpose(
                        xs[:, 1, cs:cs + 128], xbf[:, 128:256])

                for rt in range(6):
                    emit_gather(rt)

                def emit_logits_mms(rt, p):
                    """Logits matmuls for vocab chunks (2p, 2p+1) of row-tile
                    rt; p==3 also emits the target-logit gather+reduce.
                    Returns state for the deferred exp/copy emission."""
                    cs = 128 * rt
                    pse = ep.tile([128, 2, 512], F32, tag="pse")
                    for half, c in enumerate((2 * p, 2 * p + 1)):
                        for k in range(2):
                            nc.tensor.matmul(
                                pse[:, half, 0:CH], hs[:, k, cs:cs + 128],
                                sw_sb[:, k, c * CH:c * CH + CH],
                                start=(k == 0), stop=(k == 1),
                            )
                        if has_swb:
                            nc.vector.tensor_tensor(
                                out=pse[:, half, 0:CH], in0=pse[:, half, 0:CH],
                                in1=swb_sb[:, (2 * p + half) * CH:
                                           (2 * p + half) * CH + CH],
                                op=ALU.add)
                    pst = None
                    if p == 3:
                        tgi_sb = ew.tile([128, 8], I16, tag="tgi")
                        nc.gpsimd.dma_start(tgi_sb[:], tgi_d.ap()[rt])
                        pstt = ep.tile([128, 2, 512], F32, tag="pse")
                        pst = pstt[0:1, 0, 0:128]
                        for k in range(2):
                            swg = ew.tile([128, 128, 2], I16, tag="swg")
                            nc.gpsimd.ap_gather(
                                swg[:], swp_sb[:, k], tgi_sb[:],
                                channels=128, num_elems=VS, d=2, num_idxs=128,
                            )
                            mulk = ew.tile([128, 128], BF16, tag="mulk")
                            nc.vector.tensor_tensor(
                                out=mulk[:],
                                in0=swg[:].bitcast(BF16)[:, :, 0],
                                in1=hs[:, k, cs:cs + 128],
                                op=ALU.mult)
                            nc.tensor.matmul(pst, ones_bf[:, 0:1], mulk[:],
                                             start=(k == 0), stop=(k == 1))
                    return pse, pst, rt, p, cs

                def emit_exp(state):
                    pse, pst, rt, p, cs = state
                    ebuf = ew.tile([128, 2, CH], BF16, tag="ebuf")
                    nc.scalar.activation(
                        ebuf[:], pse[:, :, 0:CH], AF.Exp,
                        accum_out=se_sb[:, rt * NPAIR + p:rt * NPAIR + p + 1])
                    if pst is not None:
                        nc.scalar.copy(tg_sb[0:1, cs:cs + 128], pst)

                def emit_xwindow(m):
                    """L1 x-part + bias MMs for steps (2m, 2m+1) into a
                    fresh [128, 8chunk, 2step, 32batch] psum (one bank).
                    The bias matmuls OPEN their chunks' accumulation; the
                    per-step h-part closes each step's column slice."""
                    t0 = 2 * m
                    xw = xwp.tile([128, 8, 2, 32], F32, tag="xw")
                    for c in bch1:
                        nc.tensor.matmul(
                            xw[:, c, :, :],
                            b1x_sb[0:1, c * 128:(c + 1) * 128],
                            ones_row[0:1, 0:64], start=True, stop=False)
                    for c in range(8):
                        for kt in range(2):
                            nc.tensor.matmul(
                                xw[:, c, :, :],
                                w1_sb[:, kt, c * 128:(c + 1) * 128],
                                xs[:, kt, 32 * t0:32 * t0 + 64],
                                start=(kt == 0 and c not in bch1),
                                stop=False)
                    return xw

                def emit_l1_h(xw, t, h1T_tm1):
                    """L1(t) h-part: W-stationary matmuls into this step's
                    column slice of the x-window psum."""
                    sl = t % 2
                    for c in range(8):
                        for kt in range(2):
                            nc.tensor.matmul(
                                xw[:, c, sl, :],
                                w1_sb[:, 2 + kt, c * 128:(c + 1) * 128],
                                h1T_tm1[:, kt, :],
                                start=False, stop=(kt == 1))

                def emit_l2(t, h1T_tm1):
                    """L2(t-1) gates [128, 8, 32]; kt 0,1 = h1(t-1),
                    kt 2,3 = h2(t-2) (skipped at t==1 where h2 is zero)."""
                    psz2 = zp.tile([128, 8, 32], F32, tag="z2")
                    for c in bch2:
                        nc.tensor.matmul(
                            psz2[:, c, :],
                            b2x_sb[0:1, c * 128:(c + 1) * 128],
                            ones_row[0:1, 0:32], start=True, stop=False)
                    nkt = 2 if t == 1 else 4
                    tq0 = 32 * (t - 2)
                    for c in range(8):
                        for kt in range(nkt):
                            rhs = (h1T_tm1[:, kt, :] if kt < 2
                                   else hs[:, kt - 2, tq0:tq0 + 32])
                            nc.tensor.matmul(
                                psz2[:, c, :],
                                w2_sb[:, kt, c * 128:(c + 1) * 128],
                                rhs,
                                start=(kt == 0 and c not in bch2),
                                stop=(kt == nkt - 1))
                    return psz2

                def lstm_act(psz):
                    """One plain tanh over the transposed gate chunks
                    [i,i,o,o,j,j,f,f] (sigmoid input scales pre-folded into
                    W; f +0.5 bias pre-added in psum)."""
                    g = lw.tile([128, 8, 32], BF16, tag="g")
                    nc.scalar.activation(g[:], psz, AF.Tanh)
                    return g

                def lstm_cell(g, c_sb, hout):
                    """Cell state kept as C' = 2c.  t1' = (g_i+1)*g_j
                    (= 2*sig(i)*tanh(j)); sig(f)*C' and the add on DVE;
                    tanh(c) via the ACT input scale (0.5 * C')."""
                    t1 = lw.tile([128, 2, 32], F32, tag="t1")
                    nc.vector.scalar_tensor_tensor(
                        out=t1[:], in0=g[:, 0:2, :], scalar=1.0,
                        in1=g[:, 4:6, :], op0=ALU.add, op1=ALU.mult)
                    junk = lw.tile([128, 1], F32, tag="junk")
                    cf = lw.tile([128, 2, 32], F32, tag="cf")
                    nc.vector.affine_mul_reduce(
                        cf[:], junk[:], g[:, 6:8, :], c_sb[:], 0.5, 0.5)
                    nc.vector.tensor_tensor(out=c_sb[:], in0=cf[:],
                                            in1=t1[:], op=ALU.add)
                    tc_t = lw.tile([128, 2, 32], BF16, tag="tc")
                    nc.scalar.activation(tc_t[:], c_sb[:], AF.Tanh,
                                         scale=0.5)
                    nc.vector.affine_mul_reduce(
                        hout, junk[:], g[:, 2:4, :], tc_t[:], 0.5, 0.5)

                # ---- wavefront: slot t = L1 step t  +  L2 step t-1 ----
                h1T_prev = h0T
                xw_cur = emit_xwindow(0)
                xw_next = None
                pending_exp = None
                for t in range(T + 1):
                    ei = t - 6
                    h1T_tm1 = h1T_prev

                    # previous slot's exp first: its psum is ready, so it
                    # fills the ACT engine while this slot's MMs run
                    if pending_exp is not None:
                        emit_exp(pending_exp)
                        pending_exp = None
                    # PE fill while the chain runs
                    if ei >= 0:
                        pending_exp = emit_logits_mms(ei // 4, ei % 4)
                    # L1(t) h-part (waits on h1T(t-1); zero h at t=0)
                    if t < T:
                        emit_l1_h(xw_cur, t, h1T_tm1)
                    # L2(t-1): all inputs ready at slot start
                    psz2 = emit_l2(t, h1T_tm1) if t >= 1 else None

                    # incremental embedding gather, 6 tiles ahead
                    if t % 4 == 0 and t // 4 + 6 < RT:
                        emit_gather(t // 4 + 6)

                    # both gate ACTs first (the second runs while the first
                    # layer's DVE cell chain executes), then the cell chains
                    g1 = lstm_act(xw_cur[:, :, t % 2, :]) if t < T else None
                    g2 = lstm_act(psz2[:]) if psz2 is not None else None
                    if g1 is not None:
                        h1T = lw.tile([128, 2, 32], BF16, tag="h1T")
                        lstm_cell(g1, c1, h1T[:])
                        h1T_prev = h1T
                    if g2 is not None:
                        tp0 = 32 * (t - 1)
                        lstm_cell(g2, c2, hs[:, :, tp0:tp0 + 32])
                    # prefetch the next x window into the PE tail
                    if t % 2 == 0 and t + 2 < T:
                        xw_next = emit_xwindow(t // 2 + 1)
                    elif t % 2 == 1:
                        xw_cur = xw_next

                # trailing logits pairs
                for ei in range(T - 5, RT * NPAIR):
                    if pending_exp is not None:
                        emit_exp(pending_exp)
                    pending_exp = emit_logits_mms(ei // 4, ei % 4)
                emit_exp(pending_exp)

            nc.sync.dma_start(se_d[:], se_sb[:])
            nc.sync.dma_start(tg_d[:], tg_sb[:])

    nc.compile()
    meta = dict(T=T, V=V, n_cores=n_cores, B=B, H=H, VS=VS, BT=BT, RT=RT,
                CH=CH, NCHUNK=NCHUNK, NPAIR=NPAIR)
    return nc, meta


# ---------------- host-side prep / combine ----------------

def prep_inputs(meta, input_data, targets, embedding, W1, b1, W2, b2,
                softmax_w, softmax_b):
    """Build the per-core input maps (numpy)."""
    B, T, V = meta["B"], meta["T"], meta["V"]
    VS, RT, n_cores = meta["VS"], meta["RT"], meta["n_cores"]
    H = meta["H"]
    G4 = 4 * H

    ids_tm = np.ascontiguousarray(
        np.asarray(input_data, np.int64).T).reshape(-1)
    tgt_tm = np.ascontiguousarray(
        np.asarray(targets, np.int64).T).reshape(-1)
    ids_in = ids_tm.astype(np.int32).reshape(RT, 128, 1)

    # W column permutation [i, j, f, o] (TF order) -> [i, o, j, f], with the
    # 0.5 sigmoid input scale folded into the i/o/f columns (the device adds
    # +0.5 to the f columns in psum and does one plain tanh over all gates)
    perm = np.concatenate([
        np.arange(0, H), np.arange(3 * H, 4 * H),
        np.arange(H, 2 * H), np.arange(2 * H, 3 * H)])
    gate_scale = np.concatenate([
        np.full(2 * H, 0.5, np.float32),          # i, o
        np.ones(H, np.float32),                   # j
        np.full(H, 0.5, np.float32)])             # f

    def prep_w(W):
        Wp = (W[:, perm] * gate_scale[None, :]).astype(ml_dtypes.bfloat16)
        return np.ascontiguousarray(Wp.reshape(4, 128, G4))

    w1_in = prep_w(np.asarray(W1, np.float32))
    w2_in = prep_w(np.asarray(W2, np.float32))
    # combined per-gate bias row: perm+scaled b plus the forget +0.5
    fhalf = np.zeros(G4, np.float32)
    fhalf[3 * H:] = 0.5
    b1x = ((np.asarray(b1, np.float32)[perm] * gate_scale) + fhalf
           ).astype(ml_dtypes.bfloat16).reshape(1, G4)
    b2x = ((np.asarray(b2, np.float32)[perm] * gate_scale) + fhalf
           ).astype(ml_dtypes.bfloat16).reshape(1, G4)

    sw = np.asarray(softmax_w, np.float32)                  # [H, V]
    swb = np.asarray(softmax_b, np.float32)

    # vectorized ap_gather index layout: idx i lives at partition i%16,
    # column i//16, replicated per 16-partition group
    rtA = (np.arange(RT) * 128)[:, None, None]
    pA = (np.arange(128) % 16)[None, :, None]
    qA = (np.arange(8) * 16)[None, None, :]
    gat = rtA + qA + pA                                     # [RT, 128, 8]

    maps, masks = [], []
    for c in range(n_cores):
        shard = sw[:, c * VS:(c + 1) * VS].astype(ml_dtypes.bfloat16)
        sw_in = np.ascontiguousarray(shard.reshape(2, 128, VS))
        swi = sw_in.view(np.int16)
        swp_in = np.ascontiguousarray(
            np.stack([swi, swi], axis=-1))                  # [2,128,VS,2]

        tl = tgt_tm - c * VS
        inr = (tl >= 0) & (tl < VS)
        tlc = np.where(inr, tl, 0).astype(np.int16)
        tgi = tlc[gat]                                      # [RT, 128, 8]
        m = dict(ids=ids_in, emb=np.asarray(embedding, np.float32),
                 w1=w1_in, w2=w2_in, sw=sw_in, swp=swp_in, tgi=tgi,
                 b1x=b1x, b2x=b2x)
        if np.any(swb):
            m["swbp"] = np.ascontiguousarray(
                np.tile(swb[c * VS:(c + 1) * VS].reshape(1, VS), (128, 1)))
        maps.append(m)
        masks.append(inr.astype(np.float32))
    return maps, masks, ids_tm, tgt_tm


def combine_outputs(meta, results, masks, tgt_tm, softmax_b):
    """results: list of per-core dicts with se_out [128, RT*NPAIR] and
    tg_out [1, BT]. Returns the scalar cost (np.float32)."""
    B, T, BT = meta["B"], meta["T"], meta["BT"]
    RT, NPAIR = meta["RT"], meta["NPAIR"]
    se_all = np.zeros(BT, np.float64)
    tg_all = np.zeros(BT, np.float64)
    for c, r in enumerate(results):
        se = np.asarray(r["se_out"], np.float64)  # [128, RT*NPAIR]
        se = se.reshape(128, RT, NPAIR).sum(-1)   # [128, RT]
        se_all += se.T.reshape(-1)                # row r = rt*128 + p
        tg_all += np.asarray(r["tg_out"], np.float64)[0] * masks[c]
    tg_all += np.asarray(softmax_b, np.float64)[tgt_tm]
    loss = np.log(se_all) - tg_all
    return np.float32(loss.sum() / B / T)


# ---------------- public entry point ----------------

_CACHE = {}
last_exec_time_ns = None
last_trace_path = None


def _get_built(has_b1, has_b2, has_swb):
    key = (has_b1, has_b2, has_swb)
    if key not in _CACHE:
        _CACHE[key] = build_charrnn(T=T, V=V, n_cores=NCORES,
                                    has_b1=has_b1, has_b2=has_b2,
                                    has_swb=has_swb, num_devices=NCORES)
    return _CACHE[key]


def kernel(input_data, targets, embedding, W1, b1, W2, b2,
           softmax_w, softmax_b, _trace=False):
    global last_exec_time_ns, last_trace_path
    has_b1 = bool(np.any(np.asarray(b1)))
    has_b2 = bool(np.any(np.asarray(b2)))
    has_swb = bool(np.any(np.asarray(softmax_b)))
    nc, meta = _get_built(has_b1, has_b2, has_swb)
    maps, masks, ids_tm, tgt_tm = prep_inputs(
        meta, input_data, targets, embedding, W1, b1, W2, b2,
        softmax_w, softmax_b)
    res = run_bass_kernel_spmd(nc, maps, core_ids=list(range(NCORES)),
                               trace=_trace)
    last_exec_time_ns = res.exec_time_ns
    if res.instructions_and_trace is not None:
        last_trace_path = res.instructions_and_trace[1]
    cost = combine_outputs(meta, res.results, masks, tgt_tm, softmax_b)
    return np.asarray(cost, np.float32)

